# revision 9
# baseline (speedup 1.0000x reference)
"""Bass/Trainium2 kernel for nn_Decoder (Bahdanau-attention GRU decoder).

Sharding: pure data-parallel over batch. B=32 -> 4 batches per core x 8 cores.
Zero collectives (an 8-rank collective measures ~260us here -> unusable inside
a 64-step recurrence). Each core runs the full recurrence for its 4 batches
and writes its slice of logits/alphas.

Key restructurings (per core, 4 local batches):
- context is NEVER materialized: everything consuming context = alpha @ enc
  is precomputed against enc once on-device:
     Ua_enc = enc @ Ua.T          (attention energies)
     EW2    = enc @ W2.T          -> gx2 = alpha-weighted t'-reduce of EW2
     EMo2T  = (enc @ Mo2.T).T     -> mo2 = alphaT-matmul in the post-pass
  (W2 = gru_w_ih[:, E:], Mo2 = mo_w[:, H:3H])
- yi == si for t>=1 (1-layer GRU: output == new hidden), yi_0 = 0.
- maxout + fc run as a batched POST-PASS over all 64 steps (N=256 moving
  operand at full PE utilization); mo_w rows are host-permuted to
  [evens; odds] so maxout pool-2 becomes tensor_max of two row blocks.
- all loop weights SBUF-resident bf16; weight-stationary matmuls (FWL).

Layouts (bt is b-major: bt = b*64 + t):
  siT_bf/si_f32 [128, 32]  p=h%128, col=(h//128)*4+b
  gate psums    [128, 96]  p=row%128, col=jc*4+b   (jc = 3H//128 chunk)
  Ua_enc/EW2    [128, chunks*256]  p=out%128, col=(chunk, b, t)
  e/alpha       [1, 256]   col = b*64 + t'
  EMo2T         [64, (b, j)]  p = t'
  m             [128, (4 kc, 256 bt)]  p = m-dim%128
"""
import numpy as np
import ml_dtypes

import concourse.bass as bass
import concourse.tile as tile
import concourse.mybir as mybir
from concourse.bass_utils import run_bass_kernel_spmd

BF = mybir.dt.bfloat16
F32 = mybir.dt.float32
AF = mybir.ActivationFunctionType
ALU = mybir.AluOpType
AX = mybir.AxisListType

B, T, H, E, M, V = 32, 64, 1024, 512, 512, 32000
NB = B // 8
H2, H3, M2 = 2 * H, 3 * H, 2 * M
HC, EC, JC = H // 128, E // 128, H3 // 128   # 8, 4, 24
BT = NB * T                                   # 256
VC = V // 128                                 # 250
bf16 = ml_dtypes.bfloat16


def apx(t, offset, dims, nparts=128):
    """AP on tile t with its own partition pitch and explicit free dims."""
    return bass.AP(t.tensor, offset, [[t.ap[0][0], nparts]] + dims)


def fix_multi_waits(nc):
    """neuronxcc core_v3 codegen allows one sync wait per instruction: split
    any multi-wait instruction into preceding single-wait NOPs."""
    n = 0
    for fn in nc.m.functions:
        for bb in fn.blocks:
            il = bb.instructions
            i = 0
            while i < len(il):
                ins = il[i]
                si = getattr(ins, 'sync_info', None)
                if si is not None and si.on_wait and len(si.on_wait) > 1:
                    waits = list(si.on_wait)
                    for j, w in enumerate(waits[:-1]):
                        nop = mybir.InstNoOp(
                            name=f"{ins.name}-w{j}", engine=ins.engine,
                            sync_info=mybir.SyncInfo(on_wait=[w], on_update=[]),
                            bass_nofuse=True)
                        il.insert(i, nop)
                        i += 1
                    ins.sync_info = mybir.SyncInfo(
                        on_wait=[waits[-1]], on_update=list(si.on_update))
                    n += 1
                i += 1
    return n


def build_nc(steps=T):
    debug = steps < T
    nc = bass.Bass("TRN2", target_bir_lowering=False, debug=False,
                   num_devices=8, enable_asserts=False)

    din = {}
    for name, shape in [
            ("waT", [H, H]), ("embT", [H, E]), ("w1T", [E, H3]),
            ("whhT", [H, H3]), ("wsT", [H, H]), ("uaT", [H2, H]),
            ("w2T", [H2, H3]), ("mo2T", [H2, M2]), ("mo1T", [H, M2]),
            ("mo3T", [E, M2]), ("fcT", [M, V]), ("va", [1, H]),
            ("encT", [H2, BT]), ("h1T", [H, NB])]:
        din[name] = nc.dram_tensor(name, shape, BF, kind="ExternalInput").ap()

    d_logits = nc.dram_tensor("logits", [NB, T, V], F32,
                              kind="ExternalOutput").ap()
    d_alphas = nc.dram_tensor("alphas", [NB, T, T], F32,
                              kind="ExternalOutput").ap()
    if debug:
        d_sihist = nc.dram_tensor("sihist", [128, HC * BT], F32,
                                  kind="ExternalOutput").ap()

    with tile.TileContext(nc) as tc:
        with tc.tile_pool(name="res", bufs=1) as res:
            waT = res.tile([128, HC * H], BF, name="waT_sb")
            embT = res.tile([128, HC * E], BF, name="embT_sb")
            w1T = res.tile([128, EC * H3], BF, name="w1T_sb")
            whhT = res.tile([128, HC * H3], BF, name="whhT_sb")
            va_sb = res.tile([128, HC], BF, name="va_sb")
            uaenc = res.tile([128, HC * BT], BF, name="uaenc_sb")
            ew2 = res.tile([128, JC * BT], BF, name="ew2_sb")
            emo2T = res.tile([64, NB * M2], BF, name="emo2T_sb")
            h1T = res.tile([128, HC * NB], BF, name="h1T_sb")
            si_hist = res.tile([128, HC * BT], BF, name="si_hist")
            ye_hist = res.tile([128, EC * BT], BF, name="ye_hist")
            m_sb = res.tile([128, (M // 128) * BT], BF, name="m_sb")
            siT_bf = res.tile([128, HC * NB], BF, name="siT_bf")
            si_f32 = res.tile([128, HC * NB], F32, name="si_f32")
            q_sb = res.tile([128, HC * NB], BF, name="q_sb")
            ye_sb = res.tile([128, EC * NB], BF, name="ye_sb")
            tanh_in = res.tile([128, HC * BT], BF, name="tanh_in")
            tanh_sb = res.tile([128, HC * BT], BF, name="tanh_sb")
            mult_tmp = res.tile([128, JC * BT], BF, name="mult_tmp")
            gx2_sb = res.tile([128, JC * NB], F32, name="gx2_sb")
            alpha_f = res.tile([1, BT], F32, name="alpha_f")
            alpha_bf = res.tile([1, BT], BF, name="alpha_bf")
            alpha_bc = res.tile([128, BT], BF, name="alpha_bc")
            ones_sb = res.tile([1, 128], BF, name="ones_sb")
            mx_sb = res.tile([1, NB], F32, name="mx_sb")
            sub_sb = res.tile([1, BT], F32, name="sub_sb")
            exp_sb = res.tile([1, BT], F32, name="exp_sb")
            sm_sb = res.tile([1, NB], F32, name="sm_sb")
            rcp_sb = res.tile([1, NB], F32, name="rcp_sb")
            rz_sb = res.tile([128, 2 * HC * NB], F32, name="rz_sb")
            r_sb = res.tile([128, HC * NB], F32, name="r_sb")
            z_sb = res.tile([128, HC * NB], F32, name="z_sb")
            n1_sb = res.tile([128, HC * NB], F32, name="n1_sb")
            n2_sb = res.tile([128, HC * NB], F32, name="n2_sb")
            n_sb = res.tile([128, HC * NB], F32, name="n_sb")
            df_sb = res.tile([128, HC * NB], F32, name="df_sb")
            zd_sb = res.tile([128, HC * NB], F32, name="zd_sb")

            # ---- load resident weights ----
            for c in range(HC):
                nc.sync.dma_start(waT[:, c * H:(c + 1) * H],
                                  din["waT"][c * 128:(c + 1) * 128, :])
                nc.sync.dma_start(embT[:, c * E:(c + 1) * E],
                                  din["embT"][c * 128:(c + 1) * 128, :])
                nc.sync.dma_start(whhT[:, c * H3:(c + 1) * H3],
                                  din["whhT"][c * 128:(c + 1) * 128, :])
            for c in range(EC):
                nc.sync.dma_start(w1T[:, c * H3:(c + 1) * H3],
                                  din["w1T"][c * 128:(c + 1) * 128, :])
            nc.sync.dma_start(apx(va_sb, 0, [[1, HC]]),
                              bass.AP(din["va"].tensor, 0,
                                      [[1, 128], [128, HC]]))
            nc.sync.dma_start(apx(h1T, 0, [[NB, HC], [1, NB]]),
                              bass.AP(din["h1T"].tensor, 0,
                                      [[NB, 128], [128 * NB, HC], [1, NB]]))
            nc.gpsimd.memset(ones_sb[:], 1.0)

            # ---- pre-pass ----
            with tc.tile_pool(name="pre", bufs=1) as pre:
                encT = pre.tile([128, 16 * BT], BF, name="encT_sb")
                for c in range(16):
                    nc.sync.dma_start(encT[:, c * BT:(c + 1) * BT],
                                      din["encT"][c * 128:(c + 1) * 128, :])
                with tc.tile_pool(name="slab", bufs=2) as slabp, \
                     tc.tile_pool(name="preps", bufs=2, space="PSUM") as preps:
                    si0_ps = preps.tile([128, HC * NB], F32, name="si0_ps",
                                        tag="si0")
                    for hc in range(HC):
                        wsl = slabp.tile([128, HC * 128], BF, name="wsl",
                                         tag="wslab")
                        nc.sync.dma_start(
                            apx(wsl, 0, [[128, HC], [1, 128]]),
                            bass.AP(din["wsT"].tensor, hc * 128,
                                    [[H, 128], [128 * H, HC], [1, 128]]))
                        for kc in range(HC):
                            nc.tensor.matmul(
                                si0_ps[:, hc * NB:(hc + 1) * NB],
                                wsl[:, kc * 128:(kc + 1) * 128],
                                h1T[:, kc * NB:(kc + 1) * NB],
                                start=(kc == 0), stop=(kc == HC - 1))
                    nc.scalar.activation(si_f32[:], si0_ps[:], AF.Tanh)
                    nc.scalar.activation(siT_bf[:], si_f32[:], AF.Copy)

                    for hc in range(HC):
                        slab = slabp.tile([128, 16 * 128], BF, name="slab",
                                          tag="slab")
                        nc.sync.dma_start(
                            apx(slab, 0, [[128, 16], [1, 128]]),
                            bass.AP(din["uaT"].tensor, hc * 128,
                                    [[H, 128], [128 * H, 16], [1, 128]]))
                        ps = preps.tile([128, BT], F32, name="ps", tag="ps")
                        for kc in range(16):
                            nc.tensor.matmul(
                                ps[:], slab[:, kc * 128:(kc + 1) * 128],
                                encT[:, kc * BT:(kc + 1) * BT],
                                start=(kc == 0), stop=(kc == 15))
                        nc.scalar.activation(uaenc[:, hc * BT:(hc + 1) * BT],
                                             ps[:], AF.Copy)
                    for jc in range(JC):
                        slab = slabp.tile([128, 16 * 128], BF, name="slab",
                                          tag="slab")
                        nc.sync.dma_start(
                            apx(slab, 0, [[128, 16], [1, 128]]),
                            bass.AP(din["w2T"].tensor, jc * 128,
                                    [[H3, 128], [128 * H3, 16], [1, 128]]))
                        ps = preps.tile([128, BT], F32, name="ps", tag="ps")
                        for kc in range(16):
                            nc.tensor.matmul(
                                ps[:], slab[:, kc * 128:(kc + 1) * 128],
                                encT[:, kc * BT:(kc + 1) * BT],
                                start=(kc == 0), stop=(kc == 15))
                        nc.scalar.activation(ew2[:, jc * BT:(jc + 1) * BT],
                                             ps[:], AF.Copy)
                with tc.tile_pool(name="mo2slab", bufs=2) as mo2p, \
                     tc.tile_pool(name="emops", bufs=1, space="PSUM") as emops:
                    emo_ps = [emops.tile([64, 512], F32, name=f"emo_ps{j}")
                              for j in range(2 * NB)]
                    for kc in range(16):
                        msl = mo2p.tile([128, M2], BF, name="msl", tag="mo2sl")
                        nc.sync.dma_start(msl[:],
                                          din["mo2T"][kc * 128:(kc + 1) * 128, :])
                        for b in range(NB):
                            for sl in range(2):
                                nc.tensor.matmul(
                                    emo_ps[b * 2 + sl][:],
                                    encT[:, kc * BT + b * T:
                                         kc * BT + b * T + T],
                                    msl[:, sl * 512:(sl + 1) * 512],
                                    start=(kc == 0), stop=(kc == 15))
                    for b in range(NB):
                        for sl in range(2):
                            nc.scalar.activation(
                                emo2T[:, b * M2 + sl * 512:
                                      b * M2 + (sl + 1) * 512],
                                emo_ps[b * 2 + sl][:], AF.Copy)

            # ---- recurrence loop ----
            with tc.tile_pool(name="lps", bufs=1, space="PSUM") as lps:
                q_ps = lps.tile([128, HC * NB], F32, name="q_ps")
                ye_ps = lps.tile([128, EC * NB], F32, name="ye_ps")
                gx_ps = lps.tile([128, JC * NB], F32, name="gx_ps")
                ghn_ps = lps.tile([128, HC * NB], F32, name="ghn_ps")
                e_ps = lps.tile([1, BT], F32, name="e_ps")
                bc_ps = lps.tile([128, BT], F32, name="bc_ps")

                for t in range(steps):
                    for hc in range(HC):
                        for kc in range(HC):
                            nc.tensor.matmul(
                                q_ps[:, hc * NB:(hc + 1) * NB],
                                waT[:, kc * H + hc * 128:
                                    kc * H + (hc + 1) * 128],
                                siT_bf[:, kc * NB:(kc + 1) * NB],
                                start=(kc == 0), stop=(kc == HC - 1))
                    nc.scalar.activation(q_sb[:], q_ps[:], AF.Copy)

                    if t == 0:
                        nc.gpsimd.memset(ye_sb[:], 0.0)
                    else:
                        for ec in range(EC):
                            for kc in range(HC):
                                nc.tensor.matmul(
                                    ye_ps[:, ec * NB:(ec + 1) * NB],
                                    embT[:, kc * E + ec * 128:
                                         kc * E + (ec + 1) * 128],
                                    siT_bf[:, kc * NB:(kc + 1) * NB],
                                    start=(kc == 0), stop=(kc == HC - 1))
                        nc.scalar.activation(ye_sb[:], ye_ps[:], AF.Copy)
                    nc.vector.tensor_copy(
                        apx(ye_hist, t, [[BT, EC], [T, NB]]),
                        apx(ye_sb, 0, [[NB, EC], [1, NB]]))

                    # attention
                    nc.vector.tensor_tensor(
                        tanh_in[:], uaenc[:],
                        apx(q_sb, 0, [[NB, HC], [1, NB], [0, T]]),
                        op=ALU.add)
                    nc.scalar.activation(tanh_sb[:], tanh_in[:], AF.Tanh)
                    for hc in range(HC):
                        nc.tensor.matmul(
                            e_ps[:], va_sb[:, hc:hc + 1],
                            tanh_sb[:, hc * BT:(hc + 1) * BT],
                            start=(hc == 0), stop=(hc == HC - 1))
                    nc.vector.reduce_max(
                        mx_sb[:], apx(e_ps, 0, [[T, NB], [1, T]], 1), axis=AX.X)
                    nc.vector.tensor_tensor(
                        sub_sb[:], e_ps[:],
                        apx(mx_sb, 0, [[1, NB], [0, T]], 1), op=ALU.subtract)
                    nc.scalar.activation(exp_sb[:], sub_sb[:], AF.Exp)
                    nc.vector.reduce_sum(
                        sm_sb[:], apx(exp_sb, 0, [[T, NB], [1, T]], 1),
                        axis=AX.X)
                    nc.vector.reciprocal(rcp_sb[:], sm_sb[:])
                    nc.vector.tensor_tensor(
                        alpha_f[:], exp_sb[:],
                        apx(rcp_sb, 0, [[1, NB], [0, T]], 1), op=ALU.mult)
                    nc.sync.dma_start(
                        bass.AP(d_alphas.tensor, t * T, [[T * T, NB], [1, T]]),
                        apx(alpha_f, 0, [[T, NB], [1, T]], 1))
                    nc.scalar.activation(alpha_bf[:], alpha_f[:], AF.Copy)
                    nc.tensor.matmul(bc_ps[:], ones_sb[:], alpha_bf[:],
                                     start=True, stop=True)
                    nc.scalar.activation(alpha_bc[:], bc_ps[:], AF.Copy)

                    # gx2 = alpha-weighted reduce of EW2
                    nc.vector.tensor_tensor(
                        mult_tmp[:], ew2[:],
                        apx(alpha_bc, 0, [[0, JC], [1, BT]]), op=ALU.mult)
                    nc.vector.reduce_sum(
                        gx2_sb[:],
                        apx(mult_tmp, 0, [[BT, JC], [T, NB], [1, T]]),
                        axis=AX.X)

                    # gx1 (all 24 jc) + gh (r,z into gx_ps; n into ghn_ps)
                    for jc in range(JC):
                        last_gx1 = (jc >= 16)
                        for kc in range(EC):
                            nc.tensor.matmul(
                                gx_ps[:, jc * NB:(jc + 1) * NB],
                                w1T[:, kc * H3 + jc * 128:
                                    kc * H3 + (jc + 1) * 128],
                                ye_sb[:, kc * NB:(kc + 1) * NB],
                                start=(kc == 0),
                                stop=(last_gx1 and kc == EC - 1))
                        if jc < 16:
                            for kc in range(HC):
                                nc.tensor.matmul(
                                    gx_ps[:, jc * NB:(jc + 1) * NB],
                                    whhT[:, kc * H3 + jc * 128:
                                         kc * H3 + (jc + 1) * 128],
                                    siT_bf[:, kc * NB:(kc + 1) * NB],
                                    start=False, stop=(kc == HC - 1))
                        else:
                            for kc in range(HC):
                                nc.tensor.matmul(
                                    ghn_ps[:, (jc - 16) * NB:(jc - 15) * NB],
                                    whhT[:, kc * H3 + jc * 128:
                                         kc * H3 + (jc + 1) * 128],
                                    siT_bf[:, kc * NB:(kc + 1) * NB],
                                    start=(kc == 0), stop=(kc == HC - 1))

                    nc.vector.tensor_tensor(
                        rz_sb[:], gx_ps[:, 0:2 * HC * NB],
                        gx2_sb[:, 0:2 * HC * NB], op=ALU.add)
                    nc.scalar.activation(r_sb[:], rz_sb[:, 0:HC * NB],
                                         AF.Sigmoid)
                    nc.scalar.activation(z_sb[:], rz_sb[:, HC * NB:],
                                         AF.Sigmoid)
                    nc.vector.tensor_tensor(
                        n1_sb[:], gx_ps[:, 2 * HC * NB:],
                        gx2_sb[:, 2 * HC * NB:], op=ALU.add)
                    nc.vector.tensor_tensor(n2_sb[:], r_sb[:], ghn_ps[:],
                                            op=ALU.mult)
                    nc.vector.tensor_tensor(n_sb[:], n1_sb[:], n2_sb[:],
                                            op=ALU.add)
                    nc.scalar.activation(n_sb[:], n_sb[:], AF.Tanh)
                    nc.vector.tensor_tensor(df_sb[:], si_f32[:], n_sb[:],
                                            op=ALU.subtract)
                    nc.vector.tensor_tensor(zd_sb[:], z_sb[:], df_sb[:],
                                            op=ALU.mult)
                    nc.vector.tensor_tensor(si_f32[:], n_sb[:], zd_sb[:],
                                            op=ALU.add)
                    nc.scalar.activation(siT_bf[:], si_f32[:], AF.Copy)
                    nc.vector.tensor_copy(
                        apx(si_hist, t, [[BT, HC], [T, NB]]),
                        apx(siT_bf, 0, [[NB, HC], [1, NB]]))

            if debug:
                sih_f = res.tile([128, HC * BT], F32, name="sih_f")
                nc.vector.tensor_copy(sih_f[:], si_hist[:])
                nc.sync.dma_start(d_sihist[:, :], sih_f[:])
            else:
                # ---- post-pass: maxout + fc ----
                with tc.tile_pool(name="post", bufs=1) as post:
                    alphaT_f = post.tile([64, BT], F32, name="alphaT_f")
                    nc.sync.dma_start(
                        apx(alphaT_f, 0, [[T, NB], [1, T]], 64),
                        bass.AP(d_alphas.tensor, 0,
                                [[1, 64], [T * T, NB], [T, T]]))
                    alphaT = post.tile([64, BT], BF, name="alphaT")
                    nc.scalar.activation(alphaT[:], alphaT_f[:], AF.Copy)
                    mo1sb = post.tile([128, HC * M2], BF, name="mo1_sb")
                    mo3sb = post.tile([128, EC * M2], BF, name="mo3_sb")
                    for kc in range(HC):
                        nc.sync.dma_start(mo1sb[:, kc * M2:(kc + 1) * M2],
                                          din["mo1T"][kc * 128:(kc + 1) * 128, :])
                    for kc in range(EC):
                        nc.sync.dma_start(mo3sb[:, kc * M2:(kc + 1) * M2],
                                          din["mo3T"][kc * 128:(kc + 1) * 128, :])
                    with tc.tile_pool(name="mops", bufs=1,
                                      space="PSUM") as mops:
                        mo_ps = [mops.tile([128, BT], F32, name=f"mo_ps{j}")
                                 for j in range(8)]
                        for jc in range(8):
                            for kc in range(HC):
                                nc.tensor.matmul(
                                    mo_ps[jc][:],
                                    mo1sb[:, kc * M2 + jc * 128:
                                          kc * M2 + (jc + 1) * 128],
                                    si_hist[:, kc * BT:(kc + 1) * BT],
                                    start=(kc == 0), stop=False)
                            for kc in range(EC):
                                nc.tensor.matmul(
                                    mo_ps[jc][:],
                                    mo3sb[:, kc * M2 + jc * 128:
                                          kc * M2 + (jc + 1) * 128],
                                    ye_hist[:, kc * BT:(kc + 1) * BT],
                                    start=False, stop=False)
                            for b in range(NB):
                                nc.tensor.matmul(
                                    mo_ps[jc][:, b * T:(b + 1) * T],
                                    emo2T[:, b * M2 + jc * 128:
                                          b * M2 + (jc + 1) * 128],
                                    alphaT[:, b * T:(b + 1) * T],
                                    start=False, stop=(b == NB - 1))
                        moo = post.tile([128, 4 * BT], F32, name="moo_sb")
                        for mc in range(4):
                            nc.scalar.activation(
                                moo[:, mc * BT:(mc + 1) * BT],
                                mo_ps[mc + 4][:], AF.Copy)
                            nc.vector.tensor_tensor(
                                m_sb[:, mc * BT:(mc + 1) * BT],
                                mo_ps[mc][:], moo[:, mc * BT:(mc + 1) * BT],
                                op=ALU.max)

                    with tc.tile_pool(name="fcs", bufs=3) as fcsp, \
                         tc.tile_pool(name="fcps", bufs=2,
                                      space="PSUM") as fcps:
                        for vc in range(VC):
                            fsl = fcsp.tile([128, 4 * 128], BF, name="fsl",
                                            tag="fcslab")
                            nc.sync.dma_start(
                                apx(fsl, 0, [[128, 4], [1, 128]]),
                                bass.AP(din["fcT"].tensor, vc * 128,
                                        [[V, 128], [128 * V, 4], [1, 128]]))
                            lg = fcps.tile([128, BT], F32, name="lg", tag="lg")
                            for kc in range(4):
                                nc.tensor.matmul(
                                    lg[:], fsl[:, kc * 128:(kc + 1) * 128],
                                    m_sb[:, kc * BT:(kc + 1) * BT],
                                    start=(kc == 0), stop=(kc == 3))
                            lgs = fcsp.tile([128, BT], F32, name="lgs",
                                            tag="lgsb")
                            nc.scalar.activation(lgs[:], lg[:], AF.Copy)
                            nc.sync.dma_start(
                                bass.AP(d_logits.tensor, vc * 128,
                                        [[1, 128], [T * V, NB], [V, T]]),
                                apx(lgs, 0, [[T, NB], [1, T]]))
    fix_multi_waits(nc)
    return nc


def _prep_core_inputs(c, enc_out, hidden_enc, Ws_w, emb_w, gru_w_ih, gru_w_hh,
                      attn_Wa, attn_Ua, attn_va, mo_w, fc_w):
    bs = slice(NB * c, NB * (c + 1))
    perm = np.concatenate([np.arange(0, M2, 2), np.arange(1, M2, 2)])
    tb = lambda a: np.ascontiguousarray(a).astype(bf16)
    enc = enc_out[bs]                       # [4, 64, 2048]
    encT = enc.reshape(NB * T, H2).T        # [2048, 256] (bt b-major)
    return {
        "waT": tb(attn_Wa.T), "embT": tb(emb_w.T),
        "w1T": tb(gru_w_ih[:, :E].T), "whhT": tb(gru_w_hh.T),
        "wsT": tb(Ws_w.T), "uaT": tb(attn_Ua.T),
        "w2T": tb(gru_w_ih[:, E:].T),
        "mo2T": tb(mo_w[perm][:, H:H + H2].T),
        "mo1T": tb(mo_w[perm][:, :H].T),
        "mo3T": tb(mo_w[perm][:, H + H2:].T),
        "fcT": tb(fc_w.T), "va": tb(attn_va.reshape(1, H)),
        "encT": tb(encT), "h1T": tb(hidden_enc[1][bs].T),
    }


def kernel(enc_out, hidden_enc, Ws_w, Ws_b, emb_w, emb_b,
           gru_w_ih, gru_w_hh, gru_b_ih, gru_b_hh,
           attn_Wa, attn_Ua, attn_va, mo_w, mo_b, fc_w, fc_b,
           steps=T, _cache={}):
    args = [np.asarray(a, dtype=np.float32) for a in
            (enc_out, hidden_enc, Ws_w, emb_w, gru_w_ih, gru_w_hh,
             attn_Wa, attn_Ua, attn_va, mo_w, fc_w)]
    for nm, b in [("Ws_b", Ws_b), ("emb_b", emb_b), ("gru_b_ih", gru_b_ih),
                  ("gru_b_hh", gru_b_hh), ("mo_b", mo_b)]:
        assert not np.any(np.asarray(b)), \
            f"nonzero {nm} not supported by this kernel build"
    fc_b = np.asarray(fc_b, dtype=np.float32)

    if steps not in _cache:
        _cache[steps] = build_nc(steps)
    nc = _cache[steps]
    in_maps = [_prep_core_inputs(c, *args) for c in range(8)]
    res = run_bass_kernel_spmd(nc, in_maps, core_ids=list(range(8)))
    logits = np.concatenate([r["logits"] for r in res.results], axis=0)
    alphas = np.concatenate([r["alphas"] for r in res.results], axis=0)
    if np.any(fc_b):
        logits = logits + fc_b[None, None, :]
    if steps < T:
        return logits, alphas, [r.get("sihist") for r in res.results]
    return logits.astype(np.float32), alphas.astype(np.float32)
